# revision 24
# baseline (speedup 1.0000x reference)
"""Adaptive embedding (4-cluster masked embedding + projection) on 8 trn2 cores.

Sharding: data-parallel over the batch dim — each of the 8 NeuronCores handles
one batch row (2048 tokens); the embedding/projection tables are replicated.

Host does ROUTING only (cluster assignment, stable sort, padded index arrays);
the device gathers rows from the full tables with indirect DMA, projects
clusters 1-3 on the PE (fp32), and writes cluster-sorted output rows. The host
inverse-permutes rows into token order afterwards.

The sqrt(D_PROJ)=32 output scale is an exact power of two, so it is folded
into the emb0 table and the projection matrices bit-exactly.
"""

import os

import numpy as np

CUTOFFS = (0, 20000, 40000, 200000, 267735)
D_PROJ = 1024
DES = (1024, 256, 64, 16)
N_CORES = 8
P = 128

_BUILD_CACHE = {}
LAST_RESULT = None  # BassKernelResults of the most recent run (for profiling)


def _build(caps, vocab_sizes, mm_dtype="float32r"):
    """Build the SPMD Bass program for per-cluster tile capacities `caps`
    (number of 128-token tiles per cluster, identical on every core)."""
    import concourse.bass as bass
    import concourse.bacc as bacc
    import concourse.tile as tile
    from concourse import mybir
    from concourse.masks import make_identity

    f32 = mybir.dt.float32
    fmm = getattr(mybir.dt, mm_dtype)  # float32r: single-pass fp32 matmul
    i32 = mybir.dt.int32
    nts = list(caps)
    ntsum = sum(nts)

    nc = bacc.Bacc("TRN2", target_bir_lowering=False)
    emb = [
        nc.dram_tensor(f"emb{i}", [vocab_sizes[i], DES[i]], f32, kind="ExternalInput")
        for i in range(4)
    ]
    proj = [None] + [
        nc.dram_tensor(f"proj{i}", [DES[i], D_PROJ], f32, kind="ExternalInput")
        for i in (1, 2, 3)
    ]
    # all clusters' index columns in one tensor: one DMA, earliest gather start
    idx_all = nc.dram_tensor("idx_all", [P, ntsum], i32, kind="ExternalInput")
    out = [
        nc.dram_tensor(f"out{i}", [nts[i] * P, D_PROJ], f32, kind="ExternalOutput")
        for i in range(4)
    ]

    with tile.TileContext(nc) as tc:
        with (
            tc.tile_pool(name="const", bufs=1) as cpool,
            tc.tile_pool(name="xt", bufs=6) as xtpool,
            tc.tile_pool(name="stage", bufs=8) as spool,
            tc.tile_pool(name="tpsum", bufs=2, space="PSUM") as tppool,
            tc.tile_pool(name="mpsum", bufs=3, space="PSUM") as mpool,
        ):
            idxt_all = cpool.tile([P, ntsum], i32, name="idxt_all")
            # SWDGE load on the Pool engine itself: issues before the Sync
            # queue preamble finishes, so the first gather starts ~1us earlier
            nc.gpsimd.dma_start(out=idxt_all[:], in_=idx_all[:])
            col0 = [0, nts[0], nts[0] + nts[1], nts[0] + nts[1] + nts[2]]
            idxt = [idxt_all[:, col0[i] : col0[i] + nts[i]] for i in range(4)]

            ident = cpool.tile([P, P], f32, name="ident")
            make_identity(nc, ident)

            # Projection weights in SBUF with K on partitions. The PE consumes
            # them as float32r (single-pass fp32), which requires the SBUF
            # producer to round to f32r — stage fp32, then DVE-copy-cast.
            # proj2 first: cluster 2 is processed first.
            def load_proj_mm(name, src, rows):
                s = spool.tile([rows, D_PROJ], f32, tag="st", name=f"{name}_s")
                nc.sync.dma_start(out=s[:], in_=src)
                t = cpool.tile([rows, D_PROJ], fmm, name=name)
                nc.vector.tensor_copy(out=t[:], in_=s[:])
                return t

            p2t = load_proj_mm("p2t", proj[2][:], 64)
            p1k = [
                load_proj_mm(f"p1k{k}", proj[1][k * P : (k + 1) * P, :], P)
                for k in range(2)
            ]
            p3t = load_proj_mm("p3t", proj[3][:], 16)

            # Woven per-tile order across compute clusters: cluster 2's tiles
            # arrive gather-paced and leave PE idle gaps — spreading cluster
            # 1/3 tiles between them keeps the PE dense through the whole
            # gather phase instead of backlogging 1+3 after the gathers end.
            def weave():
                items = []
                for i in (2, 1, 3):
                    for t in range(nts[i]):
                        items.append(((t + 0.5) / nts[i], i == 2, i, t))
                items.sort(key=lambda it: (it[0], not it[1]))
                return [(i, t) for _, _, i, t in items]

            order = weave()

            # Indirect-DMA gathers. HW processes one index per partition and
            # copies out-free-size contiguous elements, so each 128-token tile
            # needs its own gather (idx column t). Cluster 0 (copy-only) last.
            g = [None] * 4
            for i in range(4):
                g[i] = cpool.tile([P, nts[i] * DES[i]], f32, name=f"g{i}")

            def gather_tile(i, ti):
                de = DES[i]
                nc.gpsimd.indirect_dma_start(
                    out=g[i][:, ti * de : (ti + 1) * de],
                    out_offset=None,
                    in_=emb[i][:],
                    in_offset=bass.IndirectOffsetOnAxis(
                        ap=idxt_all[:, col0[i] + ti : col0[i] + ti + 1], axis=0
                    ),
                )

            for i, t in order:
                gather_tile(i, t)
            for t in range(nts[0]):
                gather_tile(0, t)

            # Cluster 0 needs no projection: straight copy to DRAM.
            for t in range(nts[0]):
                nc.sync.dma_start(
                    out=out[0][t * P : (t + 1) * P, :],
                    in_=g[0][:, t * D_PROJ : (t + 1) * D_PROJ],
                )

            # Per 128-token tile: PE-transpose the gathered rows so K (=de)
            # sits on partitions (each chunk lands at partition 0 — PE needs
            # lhsT/rhs partition bases to match), project, evacuate, store.
            pws = {1: p1k, 2: [p2t], 3: [p3t]}

            def project_tile(i, t):
                de = DES[i]
                nk = (de + P - 1) // P
                pw = pws[i]
                lhs = []
                for k in range(nk):
                    w = min(P, de - k * P)
                    tp = tppool.tile([w, P], f32, tag="tp", name=f"tp{i}_{t}_{k}")
                    x = xtpool.tile([w, P], fmm, tag="xt", name=f"xt{i}_{t}_{k}")
                    lo = t * de + k * P
                    nc.tensor.transpose(
                        out=tp[:], in_=g[i][:, lo : lo + w], identity=ident[:]
                    )
                    nc.vector.tensor_copy(out=x[:], in_=tp[:])
                    lhs.append(x)
                ps = mpool.tile([P, D_PROJ], f32, tag="ps", name=f"ps{i}_{t}")
                for n in range(2):
                    for k, (lap, pwk) in enumerate(zip(lhs, pw)):
                        nc.tensor.matmul(
                            ps[:, n * 512 : (n + 1) * 512],
                            lap[:],
                            pwk[:, n * 512 : (n + 1) * 512],
                            start=(k == 0),
                            stop=(k == len(lhs) - 1),
                        )
                st = spool.tile([P, D_PROJ], f32, tag="st", name=f"st{i}_{t}")
                nc.vector.tensor_copy(out=st[:, 0:512], in_=ps[:, 0:512])
                nc.scalar.copy(out=st[:, 512:1024], in_=ps[:, 512:1024])
                nc.sync.dma_start(out=out[i][t * P : (t + 1) * P, :], in_=st[:])

            for i, t in order:
                project_tile(i, t)

    nc.compile()
    return nc


def kernel(tokens, emb0, emb1, emb2, emb3, proj1, proj2, proj3):
    global LAST_RESULT
    from concourse.bass_utils import run_bass_kernel_spmd

    toks = np.asarray(tokens).astype(np.int64, copy=False)
    nb, ns = toks.shape
    assert nb == N_CORES and ns % P == 0

    embs = [np.ascontiguousarray(np.asarray(e, dtype=np.float32)) for e in (emb0, emb1, emb2, emb3)]
    # sqrt(1024) = 32: exact power of two, folding is bit-exact.
    scale = np.float32(32.0)
    emb0s = embs[0] * scale
    projs = {
        i: np.ascontiguousarray(np.asarray(p, dtype=np.float32)) * scale
        for i, p in ((1, proj1), (2, proj2), (3, proj3))
    }

    cuts = np.asarray(CUTOFFS, dtype=np.int64)
    cluster = np.searchsorted(cuts[1:-1], toks, side="right")

    orders, counts, locs = [], [], []
    for c in range(nb):
        cl = cluster[c]
        orders.append(np.argsort(cl, kind="stable"))
        counts.append(np.bincount(cl, minlength=4))
        sizes = np.asarray([embs[i].shape[0] for i in range(4)], dtype=np.int64)
        locs.append(
            np.clip(toks[c] - cuts[cl], 0, sizes[cl] - 1).astype(np.int32)
        )
    counts = np.stack(counts)  # [nb, 4]

    caps = tuple(
        int(max(1, -(-int(counts[:, i].max()) // P))) for i in range(4)
    )  # 128-token tiles per cluster, uniform across cores
    vocab_sizes = tuple(e.shape[0] for e in embs)
    mm_dtype = os.environ.get("KERNEL_MM_DTYPE", "float32r")
    key = (caps, vocab_sizes, mm_dtype)
    if key not in _BUILD_CACHE:
        _BUILD_CACHE[key] = _build(caps, vocab_sizes, mm_dtype)
    nc = _BUILD_CACHE[key]

    in_maps = []
    for c in range(nb):
        m = {
            "emb0": emb0s,
            "emb1": embs[1],
            "emb2": embs[2],
            "emb3": embs[3],
            "proj1": projs[1],
            "proj2": projs[2],
            "proj3": projs[3],
        }
        starts = np.concatenate([[0], np.cumsum(counts[c])])
        li = locs[c][orders[c]]  # local indices, cluster-sorted
        cols = []
        for i in range(4):
            padded = np.zeros(caps[i] * P, np.int32)
            padded[: counts[c, i]] = li[starts[i] : starts[i + 1]]
            # device layout: idx[p, t] = sorted position t*128 + p
            cols.append(padded.reshape(caps[i], P).T)
        m["idx_all"] = np.ascontiguousarray(np.concatenate(cols, axis=1))
        in_maps.append(m)

    res = run_bass_kernel_spmd(nc, in_maps, core_ids=list(range(N_CORES)))
    LAST_RESULT = res

    out = np.empty((nb, ns, D_PROJ), np.float32)
    for c in range(nb):
        segs = [res.results[c][f"out{i}"][: counts[c, i]] for i in range(4)]
        out[c][orders[c]] = np.concatenate(segs, axis=0)
    return out


# revision 25
# speedup vs baseline: 1.0423x; 1.0423x over previous
"""Adaptive embedding (4-cluster masked embedding + projection) on 8 trn2 cores.

Sharding: data-parallel over the batch dim — each of the 8 NeuronCores handles
one batch row (2048 tokens); the embedding/projection tables are replicated.

Host does ROUTING only (cluster assignment, stable sort, padded index arrays);
the device gathers rows from the full tables with indirect DMA, projects
clusters 1-3 on the PE (fp32), and writes cluster-sorted output rows. The host
inverse-permutes rows into token order afterwards.

The sqrt(D_PROJ)=32 output scale is an exact power of two, so it is folded
into the emb0 table and the projection matrices bit-exactly.
"""

import os

import numpy as np

CUTOFFS = (0, 20000, 40000, 200000, 267735)
D_PROJ = 1024
DES = (1024, 256, 64, 16)
N_CORES = 8
P = 128

_BUILD_CACHE = {}
LAST_RESULT = None  # BassKernelResults of the most recent run (for profiling)


def _build(caps, vocab_sizes, mm_dtype="float32r"):
    """Build the SPMD Bass program for per-cluster tile capacities `caps`
    (number of 128-token tiles per cluster, identical on every core)."""
    import concourse.bass as bass
    import concourse.bacc as bacc
    import concourse.tile as tile
    from concourse import mybir
    from concourse.masks import make_identity

    f32 = mybir.dt.float32
    fmm = getattr(mybir.dt, mm_dtype)  # float32r: single-pass fp32 matmul
    i32 = mybir.dt.int32
    nts = list(caps)
    ntsum = sum(nts)

    nc = bacc.Bacc("TRN2", target_bir_lowering=False)
    emb = [
        nc.dram_tensor(f"emb{i}", [vocab_sizes[i], DES[i]], f32, kind="ExternalInput")
        for i in range(4)
    ]
    proj = [None] + [
        nc.dram_tensor(f"proj{i}", [DES[i], D_PROJ], f32, kind="ExternalInput")
        for i in (1, 2, 3)
    ]
    # all clusters' index columns in one tensor: one DMA, earliest gather start
    idx_all = nc.dram_tensor("idx_all", [P, ntsum], i32, kind="ExternalInput")
    out = [
        nc.dram_tensor(f"out{i}", [nts[i] * P, D_PROJ], f32, kind="ExternalOutput")
        for i in range(4)
    ]

    with tile.TileContext(nc) as tc:
        with (
            tc.tile_pool(name="const", bufs=1) as cpool,
            tc.tile_pool(name="xt", bufs=6) as xtpool,
            tc.tile_pool(name="stage", bufs=8) as spool,
            tc.tile_pool(name="tpsum", bufs=2, space="PSUM") as tppool,
            tc.tile_pool(name="mpsum", bufs=3, space="PSUM") as mpool,
        ):
            idxt_all = cpool.tile([P, ntsum], i32, name="idxt_all")
            nc.sync.dma_start(out=idxt_all[:], in_=idx_all[:])
            col0 = [0, nts[0], nts[0] + nts[1], nts[0] + nts[1] + nts[2]]
            idxt = [idxt_all[:, col0[i] : col0[i] + nts[i]] for i in range(4)]

            ident = cpool.tile([P, P], f32, name="ident")
            make_identity(nc, ident)

            # Projection weights in SBUF with K on partitions. The PE consumes
            # them as float32r (single-pass fp32), which requires the SBUF
            # producer to round to f32r — stage fp32, then DVE-copy-cast.
            # proj2 first: cluster 2 is processed first.
            def load_proj_mm(name, src, rows):
                s = spool.tile([rows, D_PROJ], f32, tag="st", name=f"{name}_s")
                nc.sync.dma_start(out=s[:], in_=src)
                t = cpool.tile([rows, D_PROJ], fmm, name=name)
                nc.vector.tensor_copy(out=t[:], in_=s[:])
                return t

            p2t = load_proj_mm("p2t", proj[2][:], 64)
            p1k = [
                load_proj_mm(f"p1k{k}", proj[1][k * P : (k + 1) * P, :], P)
                for k in range(2)
            ]
            p3t = load_proj_mm("p3t", proj[3][:], 16)

            # Woven per-tile order across compute clusters: cluster 2's tiles
            # arrive gather-paced and leave PE idle gaps — spreading cluster
            # 1/3 tiles between them keeps the PE dense through the whole
            # gather phase instead of backlogging 1+3 after the gathers end.
            def weave():
                items = []
                for i in (2, 1, 3):
                    for t in range(nts[i]):
                        items.append(((t + 0.5) / nts[i], i == 2, i, t))
                items.sort(key=lambda it: (it[0], not it[1]))
                return [(i, t) for _, _, i, t in items]

            order = weave()

            # Indirect-DMA gathers. HW processes one index per partition and
            # copies out-free-size contiguous elements, so each 128-token tile
            # needs its own gather (idx column t). Cluster 0 (copy-only) last.
            g = [None] * 4
            for i in range(4):
                g[i] = cpool.tile([P, nts[i] * DES[i]], f32, name=f"g{i}")

            def gather_tile(i, ti):
                de = DES[i]
                nc.gpsimd.indirect_dma_start(
                    out=g[i][:, ti * de : (ti + 1) * de],
                    out_offset=None,
                    in_=emb[i][:],
                    in_offset=bass.IndirectOffsetOnAxis(
                        ap=idxt_all[:, col0[i] + ti : col0[i] + ti + 1], axis=0
                    ),
                )

            for i, t in order:
                gather_tile(i, t)
            for t in range(nts[0]):
                gather_tile(0, t)

            # Cluster 0 needs no projection: straight copy to DRAM.
            for t in range(nts[0]):
                nc.sync.dma_start(
                    out=out[0][t * P : (t + 1) * P, :],
                    in_=g[0][:, t * D_PROJ : (t + 1) * D_PROJ],
                )

            # Per 128-token tile: PE-transpose the gathered rows so K (=de)
            # sits on partitions (each chunk lands at partition 0 — PE needs
            # lhsT/rhs partition bases to match), project, evacuate, store.
            pws = {1: p1k, 2: [p2t], 3: [p3t]}

            def project_tile(i, t):
                de = DES[i]
                nk = (de + P - 1) // P
                pw = pws[i]
                lhs = []
                for k in range(nk):
                    w = min(P, de - k * P)
                    tp = tppool.tile([w, P], f32, tag="tp", name=f"tp{i}_{t}_{k}")
                    x = xtpool.tile([w, P], fmm, tag="xt", name=f"xt{i}_{t}_{k}")
                    lo = t * de + k * P
                    nc.tensor.transpose(
                        out=tp[:], in_=g[i][:, lo : lo + w], identity=ident[:]
                    )
                    nc.vector.tensor_copy(out=x[:], in_=tp[:])
                    lhs.append(x)
                ps = mpool.tile([P, D_PROJ], f32, tag="ps", name=f"ps{i}_{t}")
                for n in range(2):
                    for k, (lap, pwk) in enumerate(zip(lhs, pw)):
                        nc.tensor.matmul(
                            ps[:, n * 512 : (n + 1) * 512],
                            lap[:],
                            pwk[:, n * 512 : (n + 1) * 512],
                            start=(k == 0),
                            stop=(k == len(lhs) - 1),
                        )
                st = spool.tile([P, D_PROJ], f32, tag="st", name=f"st{i}_{t}")
                nc.vector.tensor_copy(out=st[:, 0:512], in_=ps[:, 0:512])
                nc.scalar.copy(out=st[:, 512:1024], in_=ps[:, 512:1024])
                nc.sync.dma_start(out=out[i][t * P : (t + 1) * P, :], in_=st[:])

            for i, t in order:
                project_tile(i, t)

    nc.compile()
    return nc


def kernel(tokens, emb0, emb1, emb2, emb3, proj1, proj2, proj3):
    global LAST_RESULT
    from concourse.bass_utils import run_bass_kernel_spmd

    toks = np.asarray(tokens).astype(np.int64, copy=False)
    nb, ns = toks.shape
    assert nb == N_CORES and ns % P == 0

    embs = [np.ascontiguousarray(np.asarray(e, dtype=np.float32)) for e in (emb0, emb1, emb2, emb3)]
    # sqrt(1024) = 32: exact power of two, folding is bit-exact.
    scale = np.float32(32.0)
    emb0s = embs[0] * scale
    projs = {
        i: np.ascontiguousarray(np.asarray(p, dtype=np.float32)) * scale
        for i, p in ((1, proj1), (2, proj2), (3, proj3))
    }

    cuts = np.asarray(CUTOFFS, dtype=np.int64)
    cluster = np.searchsorted(cuts[1:-1], toks, side="right")

    orders, counts, locs = [], [], []
    for c in range(nb):
        cl = cluster[c]
        orders.append(np.argsort(cl, kind="stable"))
        counts.append(np.bincount(cl, minlength=4))
        sizes = np.asarray([embs[i].shape[0] for i in range(4)], dtype=np.int64)
        locs.append(
            np.clip(toks[c] - cuts[cl], 0, sizes[cl] - 1).astype(np.int32)
        )
    counts = np.stack(counts)  # [nb, 4]

    caps = tuple(
        int(max(1, -(-int(counts[:, i].max()) // P))) for i in range(4)
    )  # 128-token tiles per cluster, uniform across cores
    vocab_sizes = tuple(e.shape[0] for e in embs)
    mm_dtype = os.environ.get("KERNEL_MM_DTYPE", "float32r")
    key = (caps, vocab_sizes, mm_dtype)
    if key not in _BUILD_CACHE:
        _BUILD_CACHE[key] = _build(caps, vocab_sizes, mm_dtype)
    nc = _BUILD_CACHE[key]

    in_maps = []
    for c in range(nb):
        m = {
            "emb0": emb0s,
            "emb1": embs[1],
            "emb2": embs[2],
            "emb3": embs[3],
            "proj1": projs[1],
            "proj2": projs[2],
            "proj3": projs[3],
        }
        starts = np.concatenate([[0], np.cumsum(counts[c])])
        li = locs[c][orders[c]]  # local indices, cluster-sorted
        cols = []
        for i in range(4):
            padded = np.zeros(caps[i] * P, np.int32)
            padded[: counts[c, i]] = li[starts[i] : starts[i + 1]]
            # device layout: idx[p, t] = sorted position t*128 + p
            cols.append(padded.reshape(caps[i], P).T)
        m["idx_all"] = np.ascontiguousarray(np.concatenate(cols, axis=1))
        in_maps.append(m)

    res = run_bass_kernel_spmd(nc, in_maps, core_ids=list(range(N_CORES)))
    LAST_RESULT = res

    out = np.empty((nb, ns, D_PROJ), np.float32)
    for c in range(nb):
        segs = [res.results[c][f"out{i}"][: counts[c, i]] for i in range(4)]
        out[c][orders[c]] = np.concatenate(segs, axis=0)
    return out
